# revision 33
# baseline (speedup 1.0000x reference)
"""Trainium2 Bass kernel for EnhancedOFTQKVLayer.

Computes out[b,s,o] = x[b,s,:] @ filt[o,:]^T + bias[o] where
filt = [Wq @ BD(cayley(q_R)); Wk @ BD(cayley(k_R)); Wv @ BD(cayley(v_R))]
(BD = block-diagonal, cayley(A) = (I-S) inv(I+S+eps I), S = 0.5(A-A^T)).

Distribution: data-parallel - batch b (8 rows) sharded one per NeuronCore;
attn_weight / bias / rotation matrices replicated.

Per-core schedule (v6):
  1. ALL matmul-operand transposes are done by DMA, not compute engines:
     x and W are cast fp32->bf16 straight in DRAM by SWDGE cast-DMAs
     (gpsimd queue, ordered by deadline) and pulled into SBUF
     pre-transposed by 1 MB DMA-xbar loads.  Zero PE/DVE/ScalarE cost.
  2. Cayley via SPD Newton-Schulz on P = (1+eps)^2 I - S^2 (iterates are
     polynomials in S^2, hence symmetric -> lhsT=operand works without
     transposes).  X0 = aI + bP with (a, b) the true minimax-residual
     linear init on the measured spectrum [1, 254] - one fewer fp16
     iteration than the generic init for the same residual.  5 fp16
     iterations (symmetrize on 2 and 4) + 2 fp32 polish iterations; the
     polish runs pairwise q->k->v so the q-projection finishes first and
     the polish of k/v overlaps the phase-1 GEMM.  rmat is pre-permuted
     on the host so one contiguous DMA loads all rotation blocks.
  3. Main matmul in bf16 (fp32 PSUM), three phases: (1) og0 then og1 over
     the first 12 row tiles, interleaved with the k/v polish and the
     og2-5 filtT builds, (2) row-tile-outer over the remaining 20 tiles x
     all 6 og groups (x^T streams through a 3-slot ring), (3) og2-5 for
     the first 12 row tiles (pure GEMM tail, x^T sg0-2 still resident).
     Fused bias add on DVE, 512 KB output DMAs on the ACT queue.
"""

import numpy as np

import concourse.bass as bass
import concourse.mybir as mybir
import concourse.tile as tile
from concourse import bacc
from concourse.bass import ds, ts
from concourse.masks import make_identity
from concourse.bass_utils import run_bass_kernel_spmd

F32 = mybir.dt.float32
F16 = mybir.dt.float16
BF16 = mybir.dt.bfloat16

MAIN_DT = BF16           # dtype of the big matmul inputs (x, filtT)

HIDDEN = 1024
OUT_DIM = 3 * HIDDEN
SEQ = 4096
P = 128
NBLK = 8                 # 128-blocks per hidden
NROT = 24                # 3 * NBLK rotation blocks
EPS = 1e-6
N_CORES = 8

NSETS = 6                # Newton processes blocks in sets of 4
SETB = 4

# Newton-Schulz schedule (validated offline against the jax reference:
# max block rel-err 5.0e-3 vs 8.8e-3 for the older 7+1 schedule).
NEWTON_F16 = 5
NEWTON_F32 = 2
SYM_ITERS = {4}          # symmetrize on these fp16 iterations
X0_A = 3.0874517e-02     # X0 = aI + bP (minimax residual on [1, 254])
X0_B = -1.2101700e-04

M_TILES = SEQ // P       # 32 row tiles of 128
SG = SEQ // 512          # 8 row groups of 512 (4 row tiles each)
O_TILES = OUT_DIM // 512  # 6
N1 = 12                  # phase-1 row tiles (og0/og1 early wave)


def build_body(ctx, tc):
    nc = tc.nc

    x = nc.dram_tensor("x", [SEQ, HIDDEN], F32, kind="ExternalInput").ap()
    w = nc.dram_tensor("w", [OUT_DIM, HIDDEN], F32, kind="ExternalInput").ap()
    bias = nc.dram_tensor("bias", [OUT_DIM], F32, kind="ExternalInput").ap()
    # host-side pre-permuted rotations: rmt[p, n, c] = rmat[n, p, c]
    rmt_d = nc.dram_tensor("rmt", [P, NROT, P], F32, kind="ExternalInput").ap()
    out = nc.dram_tensor("out", [SEQ, OUT_DIM], F32, kind="ExternalOutput").ap()

    sub = nc.vector.tensor_sub
    add = nc.vector.tensor_add
    smul = nc.vector.tensor_scalar_mul
    cp = nc.vector.tensor_copy
    scp = nc.scalar.copy

    def bc(t):  # broadcast a [P, P] constant over a set's middle dim
        return t[:].unsqueeze(1).to_broadcast([P, SETB, P])

    # ---- persistent pools ----
    const = ctx.enter_context(tc.tile_pool(name="const", bufs=1))
    ftp = ctx.enter_context(tc.tile_pool(name="ftp", bufs=1))
    qpool = ctx.enter_context(tc.tile_pool(name="qpool", bufs=1))
    dram = ctx.enter_context(tc.tile_pool(name="dram", bufs=1, space="DRAM"))

    ident32 = const.tile([P, P], F32)
    make_identity(nc, ident32)
    eI2 = const.tile([P, P], F32)       # (1+eps)^2 I
    smul(eI2[:], ident32[:], float((1.0 + EPS) ** 2))
    eI12 = const.tile([P, P], F32)      # ((1+eps) + (1+eps)^2) I
    smul(eI12[:], ident32[:], float((1.0 + EPS) + (1.0 + EPS) ** 2))
    twoI = const.tile([P, P], F32)      # 2 I
    smul(twoI[:], ident32[:], 2.0)
    aI0 = const.tile([P, P], F16)       # X0_A * I  (Newton init)
    smul(aI0[:], ident32[:], float(X0_A))

    # bf16 DRAM scratch (filled by SWDGE cast-DMAs, deadline order)
    xb = dram.tile([SEQ, HIDDEN], MAIN_DT)
    wb = dram.tile([OUT_DIM, HIDDEN], MAIN_DT)
    bias_bc = const.tile([P, OUT_DIM], MAIN_DT)
    with tc.tile_pool(name="biasld", bufs=1) as bl:
        brow = bl.tile([1, OUT_DIM], F32)
        nc.sync.dma_start(brow[:], bias.unsqueeze(0))
        cp(bias_bc[:1, :], brow[:])

    # filtT chunks: ft[k][og][c, o'] = filtT[k*128+c, og*512+o']
    ft = [[ftp.tile([P, 512], MAIN_DT, tag=f"ft{k}_{og}", name=f"ft{k}_{og}")
           for og in range(O_TILES)] for k in range(NBLK)]

    with (
        tc.tile_pool(name="nper", bufs=1) as nper,     # per-set persistents
        tc.tile_pool(name="nx", bufs=1) as nxp,        # per-set iterates
        tc.tile_pool(name="nu", bufs=4) as nup,        # U temp
        tc.tile_pool(name="misc", bufs=1) as misc,
        tc.tile_pool(name="wtsp", bufs=3) as wtsp,     # W^T og ring
        tc.tile_pool(name="xt01", bufs=1) as xt01p,    # x^T sg0-2 resident
        tc.tile_pool(name="xtp", bufs=2) as xtp,       # x^T sg3-7 ring
        tc.tile_pool(name="obp", bufs=3) as obp,       # out staging
        tc.tile_pool(name="ps_g", bufs=4, space="PSUM") as ps_g,
        tc.tile_pool(name="ps_out", bufs=4, space="PSUM") as ps_out,
    ):
        # ---- W^T / x^T via DMA-xbar transposed loads ----
        # The SWDGE casts are self-paced: cast k is released (via a tiny
        # dummy WAW write into its destination range) by the completion of
        # the xbar TP two chain positions earlier, so at most two casts
        # are in flight and they complete in deadline order instead of
        # round-robining to a common late finish.
        CH = [("w", 0), ("x", 0), ("w", 1), ("x", 1), ("w", 2), ("x", 2),
              ("w", 3), ("x", 3), ("w", 4), ("w", 5), ("x", 4), ("x", 5),
              ("x", 6), ("x", 7)]
        tp_tiles = []          # TP output tiles in chain order
        wts = {}
        xts = {}

        def release_cast(k, src):
            kind, idx = CH[k]
            if kind == "w":
                nc.sync.dma_start(wb[ds(idx * 512, 1), :16], src)
                nc.gpsimd.dma_start(wb[ts(idx, 512), :], w[ts(idx, 512), :])
            else:
                nc.sync.dma_start(xb[ds(idx * 512, 1), :16], src)
                nc.gpsimd.dma_start(xb[ts(idx, 512), :], x[ts(idx, 512), :])

        def _chain_next(tile):
            tp_tiles.append(tile)
            k = len(tp_tiles) + 1
            if k < len(CH):
                release_cast(k, tile[:1, 0, :16])

        def emit_wT(og):
            wts[og] = wtsp.tile([P, NBLK, 512], MAIN_DT, tag="wts",
                                name=f"wts{og}")
            nc.sync.dma_start(wts[og][:], wb[ts(og, 512), :], transpose=True)
            _chain_next(wts[og])

        def emit_xt(sg):
            pool, tag = (xt01p, f"xt{sg}") if sg < 3 else (xtp, "xt")
            t = pool.tile([P, NBLK, 512], MAIN_DT, tag=tag, name=f"xt{sg}")
            nc.sync.dma_start(t[:], xb[ts(sg, 512), :], transpose=True)
            xts[sg] = t
            _chain_next(t)

        # ------- rotations + S prep (rmt freed right after) -------
        # The SWDGE casts are gated behind the rmt arrival via tiny dummy
        # writes into each cast's destination range (WAW dep), so their
        # ~30 MB of HBM traffic cannot starve the latency-critical loads.
        s_s = []
        with tc.tile_pool(name="rmt", bufs=1) as rmtp:
            rmt = rmtp.tile([P, NROT, P], F32)
            nc.sync.dma_start(rmt[:], rmt_d)
            trig = misc.tile([1, 16], MAIN_DT, tag="trig")
            cp(trig[:], rmt[:1, 0, :16])
            release_cast(0, trig[:])
            release_cast(1, trig[:])
            nc.gpsimd.partition_broadcast(bias_bc[:], bias_bc[:1, :])

            emit_wT(0)
            emit_xt(0)
            emit_wT(1)
            emit_xt(1)
            for s in range(NSETS):
                tpg = ps_g.tile([P, SETB, P], F32, tag="g")
                for j in range(SETB):
                    nc.tensor.transpose(tpg[:, j, :], rmt[:, s * SETB + j, :],
                                        ident32[:])
                sset = nper.tile([P, SETB, P], F32, tag=f"s{s}", name=f"s{s}")
                for j in range(SETB):
                    # Su = A - A^T (unscaled; the 1/2 folds into S^2 and B^T)
                    sub(sset[:, j, :], rmt[:, s * SETB + j, :], tpg[:, j, :])
                s_s.append(sset)

        # ---------- Newton-Cayley: 6 interleaved sets of 4 blocks ----------
        stt = nc.vector.scalar_tensor_tensor
        p32_s, p16_s, x_s = [], [], []
        for s in range(NSETS):
            g = ps_g.tile([P, SETB, P], F32, tag="g")
            for j in range(SETB):                # Su^T @ Su = -4 S^2
                nc.tensor.matmul(g[:, j, :], lhsT=s_s[s][:, j, :],
                                 rhs=s_s[s][:, j, :], start=True, stop=True)
            p32s = nper.tile([P, SETB, P], F32, tag=f"p32{s}", name=f"p32{s}")
            # P = (1+e)^2 I - S^2 = 0.25 * (Su^T Su) + (1+e)^2 I
            stt(p32s[:], g[:], 0.25, bc(eI2),
                mybir.AluOpType.mult, mybir.AluOpType.add)
            p16s = nper.tile([P, SETB, P], F16, tag=f"p16{s}", name=f"p16{s}")
            scp(p16s[:], p32s[:])
            xset = nxp.tile([P, SETB, P], F16, tag=f"x{s}", name=f"x{s}_init")
            # X0 = aI + bP
            stt(xset[:], p32s[:], float(X0_B), bc(aI0),
                mybir.AluOpType.mult, mybir.AluOpType.add)
            # fold B^T = eI12 + ((2+e)/2) Su - P into the S tile now
            stt(s_s[s][:], s_s[s][:], float((2.0 + EPS) / 2.0), p32s[:],
                mybir.AluOpType.mult, mybir.AluOpType.subtract)
            add(s_s[s][:], s_s[s][:], bc(eI12))
            p32_s.append(p32s)
            p16_s.append(p16s)
            x_s.append(xset)

        for i in range(NEWTON_F16):
            do_sym = i in SYM_ITERS
            for s in range(NSETS):
                # one PSUM tile per (iter, set): g1, then g2 overwrites it
                # (the DVE read of g1 gates the g2 matmuls anyway).
                g = ps_g.tile([P, SETB, P], F32, tag="g")
                for j in range(SETB):
                    nc.tensor.matmul(g[:, j, :], lhsT=p16_s[s][:, j, :],
                                     rhs=x_s[s][:, j, :], start=True,
                                     stop=True)
                u = nup.tile([P, SETB, P], F16, tag="u")
                sub(u[:], bc(twoI), g[:])                # U = 2I - P X (DVE)
                for j in range(SETB):                    # X' = X U
                    nc.tensor.matmul(g[:, j, :], lhsT=x_s[s][:, j, :],
                                     rhs=u[:, j, :], start=True, stop=True)
                xset = nxp.tile([P, SETB, P], F16, tag=f"x{s}",
                                name=f"x{s}_{i}")
                if not do_sym:
                    scp(xset[:], g[:])                   # ScalarE
                else:
                    xc = misc.tile([P, SETB, P], F32, tag="xc")
                    scp(xc[:], g[:])
                    tpg = ps_g.tile([P, SETB, P], F32, tag="g")
                    for j in range(SETB):
                        nc.tensor.transpose(tpg[:, j, :], xc[:, j, :],
                                            ident32[:])
                    add(xc[:], xc[:], tpg[:])
                    nc.scalar.activation(xset[:], xc[:],
                                         mybir.ActivationFunctionType.Copy,
                                         scale=0.5)
                x_s[s] = xset

        # fp32 polish + Q for a pair of sets (q first, then k, then v)
        q_s = [None] * NSETS

        def polish_q(pair):
            xfs = {}
            for s in pair:
                xf = nxp.tile([P, SETB, P], F32, tag=f"xf{s % 2}",
                              name=f"xf{s}")
                if s % 2 == 0:
                    cp(xf[:], x_s[s][:])
                else:
                    scp(xf[:], x_s[s][:])
                xfs[s] = xf
            for i in range(NEWTON_F32):
                for s in pair:
                    g = ps_g.tile([P, SETB, P], F32, tag="g")
                    for j in range(SETB):
                        nc.tensor.matmul(g[:, j, :], lhsT=p32_s[s][:, j, :],
                                         rhs=xfs[s][:, j, :], start=True,
                                         stop=True)
                    uf = misc.tile([P, SETB, P], F32, tag=f"uf{s % 2}")
                    sub(uf[:], bc(twoI), g[:])
                    for j in range(SETB):
                        nc.tensor.matmul(g[:, j, :], lhsT=xfs[s][:, j, :],
                                         rhs=uf[:, j, :], start=True,
                                         stop=True)
                    xf2 = nxp.tile([P, SETB, P], F32, tag=f"xf{s % 2}",
                                   name=f"xf{s}_{i}")
                    if s % 2 == 0:
                        cp(xf2[:], g[:])
                    else:
                        scp(xf2[:], g[:])
                    xfs[s] = xf2
            for s in pair:
                # Q = B @ X with B^T = eI12 + (2+e)S - P (folded into s_s)
                g = ps_g.tile([P, SETB, P], F32, tag="g")
                for j in range(SETB):
                    nc.tensor.matmul(g[:, j, :], lhsT=s_s[s][:, j, :],
                                     rhs=xfs[s][:, j, :], start=True,
                                     stop=True)
                qset = qpool.tile([P, SETB, P], MAIN_DT, tag=f"q{s}",
                                  name=f"q{s}")
                if s % 2 == 0:
                    cp(qset[:], g[:])
                else:
                    scp(qset[:], g[:])
                q_s[s] = qset

        def q_lhsT(n):
            return q_s[n // SETB][:, n % SETB, :]

        # ---- filtT chunks: ft[k][og] = Q^T W^T ----
        def emit_ft(og):
            part = og // 2             # q/k/v
            for k in range(NBLK):
                fg = ps_out.tile([P, 512], F32, tag="po", name=f"fg{og}_{k}")
                nc.tensor.matmul(fg[:], lhsT=q_lhsT(part * NBLK + k),
                                 rhs=wts[og][:, k, :], start=True, stop=True)
                if k % 2 == 0:
                    cp(ft[k][og][:], fg[:])
                else:
                    scp(ft[k][og][:], fg[:])

        # ---- main GEMM bursts ----
        def emit_gemm(mt, og_lo, n_og):
            sg, sb = mt // 4, mt % 4
            ob = obp.tile([P, 512 * n_og], F32, tag="ob",
                          name=f"ob{mt}_{og_lo}")
            for h in range(n_og):
                og = og_lo + h
                po = ps_out.tile([P, 512], F32, tag="po", name=f"po{mt}_{og}")
                for k in range(NBLK):
                    nc.tensor.matmul(po[:], lhsT=xts[sg][:, k, ts(sb, P)],
                                     rhs=ft[k][og][:],
                                     start=(k == 0), stop=(k == NBLK - 1))
                add(ob[:, ts(h, 512)], po[:], bias_bc[:, ts(og, 512)])
            nc.scalar.dma_start(
                out[ts(mt, P), ds(og_lo * 512, 512 * n_og)], ob[:])

        # ================= tail emission order (= priority) =============
        polish_q((0, 1))
        emit_ft(0)
        emit_ft(1)
        emit_wT(2)
        emit_xt(2)
        # phase 1: og0 then og1 over the first N1 row tiles; k/v polish and
        # og2-5 filtT builds slot into the stream between chunks.
        for mt in range(N1):
            emit_gemm(mt, 0, 1)
            if mt == 0:
                polish_q((2, 3))
            if mt == 2:
                emit_ft(2)
            if mt == 5:
                emit_wT(3)
            if mt == 8:
                emit_ft(3)
        emit_xt(3)
        for mt in range(N1):
            emit_gemm(mt, 1, 1)
            if mt == 0:
                polish_q((4, 5))
            if mt == 2:
                emit_wT(4)
            if mt == 4:
                emit_ft(4)
            if mt == 6:
                emit_wT(5)
            if mt == 8:
                emit_ft(5)
        emit_xt(4)
        # phase 2: remaining row tiles, all og groups
        for mt in range(N1, M_TILES):
            if mt % 4 == 0 and mt + 8 < M_TILES:
                emit_xt((mt + 8) // 4)      # prefetch sg two groups ahead
            for pair in range(3):
                emit_gemm(mt, 2 * pair, 2)
        # phase 3: og2-5 for the first N1 row tiles (x^T still resident)
        for mt in range(N1):
            emit_gemm(mt, 2, 2)
            emit_gemm(mt, 4, 2)


def build():
    if "nc" in _CACHE:
        return _CACHE["nc"]
    import contextlib

    nc = bacc.Bacc("TRN2", target_bir_lowering=False, debug=False)
    with tile.TileContext(nc) as tc:
        with contextlib.ExitStack() as ctx:
            build_body(ctx, tc)
    nc.compile()
    _CACHE["nc"] = nc
    return nc


_CACHE = {}


def make_in_maps(attn_weight, bias, x, q_R, k_R, v_R):
    rmat = np.concatenate([q_R, k_R, v_R], axis=0).astype(np.float32)
    rmt = np.ascontiguousarray(rmat.transpose(1, 0, 2))  # [P, NROT, P]
    w = np.ascontiguousarray(attn_weight, dtype=np.float32)
    b = np.ascontiguousarray(bias, dtype=np.float32)
    return [
        {"x": np.ascontiguousarray(x[c], dtype=np.float32),
         "w": w, "bias": b, "rmt": rmt}
        for c in range(N_CORES)
    ]


def kernel(attn_weight, bias, x, q_R, k_R, v_R, **run_kwargs):
    nc = build()
    in_maps = make_in_maps(attn_weight, bias, x, q_R, k_R, v_R)
    res = run_bass_kernel_spmd(nc, in_maps, core_ids=list(range(N_CORES)),
                               **run_kwargs)
    out = np.stack([res.results[c]["out"] for c in range(N_CORES)], axis=0)
    _CACHE["last_results"] = res
    return out


# revision 34
# speedup vs baseline: 1.0392x; 1.0392x over previous
"""Trainium2 Bass kernel for EnhancedOFTQKVLayer.

Computes out[b,s,o] = x[b,s,:] @ filt[o,:]^T + bias[o] where
filt = [Wq @ BD(cayley(q_R)); Wk @ BD(cayley(k_R)); Wv @ BD(cayley(v_R))]
(BD = block-diagonal, cayley(A) = (I-S) inv(I+S+eps I), S = 0.5(A-A^T)).

Distribution: data-parallel - batch b (8 rows) sharded one per NeuronCore;
attn_weight / bias / rotation matrices replicated.

Per-core schedule:
  1. ALL matmul-operand transposes are done by DMA, not compute engines:
     x and W are cast fp32->bf16 straight in DRAM by SWDGE cast-DMAs
     (gpsimd queue, ordered by deadline) and pulled into SBUF
     pre-transposed by 1 MB DMA-xbar loads.  Zero PE/DVE/ScalarE cost.
  2. Cayley via SPD Newton-Schulz on P = (1+eps)^2 I - S^2 (iterates are
     polynomials in S^2, hence symmetric -> lhsT=operand works without
     transposes).  X0 = aI + bP with (a, b) the true minimax-residual
     linear init on the measured spectrum [1, 254] - one fewer fp16
     iteration than the generic init for the same residual.  5 fp16
     iterations (symmetrize on 2 and 4) + 2 fp32 polish iterations; the
     polish runs pairwise q->k->v so the q-projection finishes first and
     the polish of k/v overlaps the phase-1 GEMM.  rmat is pre-permuted
     on the host so one contiguous DMA loads all rotation blocks.
  3. Main matmul in bf16 (fp32 PSUM), three phases: (1) og0 then og1 over
     the first 12 row tiles, interleaved with the k/v polish and the
     og2-5 filtT builds, (2) row-tile-outer over the remaining 20 tiles x
     all 6 og groups (x^T streams through a 2-slot ring), (3) og2-5 for
     the first 12 row tiles (pure GEMM tail, x^T sg0-2 still resident).
     Fused bias add on DVE, 512 KB output DMAs on the ACT queue.
"""

import numpy as np

import concourse.bass as bass
import concourse.mybir as mybir
import concourse.tile as tile
from concourse import bacc
from concourse.bass import ds, ts
from concourse.masks import make_identity
from concourse.bass_utils import run_bass_kernel_spmd

F32 = mybir.dt.float32
F16 = mybir.dt.float16
BF16 = mybir.dt.bfloat16

MAIN_DT = BF16           # dtype of the big matmul inputs (x, filtT)

HIDDEN = 1024
OUT_DIM = 3 * HIDDEN
SEQ = 4096
P = 128
NBLK = 8                 # 128-blocks per hidden
NROT = 24                # 3 * NBLK rotation blocks
EPS = 1e-6
N_CORES = 8

NSETS = 6                # Newton processes blocks in sets of 4
SETB = 4

# Newton-Schulz schedule (validated offline against the jax reference:
# max block rel-err 5.0e-3 vs 8.8e-3 for the older 7+1 schedule).
NEWTON_F16 = 5
NEWTON_F32 = 2
SYM_ITERS = {2, 4}       # symmetrize on these fp16 iterations
X0_A = 3.0874517e-02     # X0 = aI + bP (minimax residual on [1, 254])
X0_B = -1.2101700e-04

M_TILES = SEQ // P       # 32 row tiles of 128
SG = SEQ // 512          # 8 row groups of 512 (4 row tiles each)
O_TILES = OUT_DIM // 512  # 6
N1 = 12                  # phase-1 row tiles (og0/og1 early wave)


def build_body(ctx, tc):
    nc = tc.nc

    x = nc.dram_tensor("x", [SEQ, HIDDEN], F32, kind="ExternalInput").ap()
    w = nc.dram_tensor("w", [OUT_DIM, HIDDEN], F32, kind="ExternalInput").ap()
    bias = nc.dram_tensor("bias", [OUT_DIM], F32, kind="ExternalInput").ap()
    # host-side pre-permuted rotations: rmt[p, n, c] = rmat[n, p, c]
    rmt_d = nc.dram_tensor("rmt", [P, NROT, P], F32, kind="ExternalInput").ap()
    out = nc.dram_tensor("out", [SEQ, OUT_DIM], F32, kind="ExternalOutput").ap()

    sub = nc.vector.tensor_sub
    add = nc.vector.tensor_add
    smul = nc.vector.tensor_scalar_mul
    cp = nc.vector.tensor_copy
    scp = nc.scalar.copy

    def bc(t):  # broadcast a [P, P] constant over a set's middle dim
        return t[:].unsqueeze(1).to_broadcast([P, SETB, P])

    # ---- persistent pools ----
    const = ctx.enter_context(tc.tile_pool(name="const", bufs=1))
    ftp = ctx.enter_context(tc.tile_pool(name="ftp", bufs=1))
    qpool = ctx.enter_context(tc.tile_pool(name="qpool", bufs=1))
    dram = ctx.enter_context(tc.tile_pool(name="dram", bufs=1, space="DRAM"))

    ident32 = const.tile([P, P], F32)
    make_identity(nc, ident32)
    eI2 = const.tile([P, P], F32)       # (1+eps)^2 I
    smul(eI2[:], ident32[:], float((1.0 + EPS) ** 2))
    eI12 = const.tile([P, P], F32)      # ((1+eps) + (1+eps)^2) I
    smul(eI12[:], ident32[:], float((1.0 + EPS) + (1.0 + EPS) ** 2))
    twoI = const.tile([P, P], F32)      # 2 I
    smul(twoI[:], ident32[:], 2.0)
    aI0 = const.tile([P, P], F16)       # X0_A * I  (Newton init)
    smul(aI0[:], ident32[:], float(X0_A))

    # bf16 DRAM scratch (filled by SWDGE cast-DMAs, deadline order)
    xb = dram.tile([SEQ, HIDDEN], MAIN_DT)
    wb = dram.tile([OUT_DIM, HIDDEN], MAIN_DT)

    nc.gpsimd.dma_start(wb[ts(0, 512), :], w[ts(0, 512), :])
    nc.gpsimd.dma_start(xb[ts(0, 512), :], x[ts(0, 512), :])
    nc.gpsimd.dma_start(wb[ts(1, 512), :], w[ts(1, 512), :])
    nc.gpsimd.dma_start(xb[ts(1, 512), :], x[ts(1, 512), :])

    bias_bc = const.tile([P, OUT_DIM], MAIN_DT)
    with tc.tile_pool(name="biasld", bufs=1) as bl:
        brow = bl.tile([1, OUT_DIM], F32)
        nc.sync.dma_start(brow[:], bias.unsqueeze(0))
        cp(bias_bc[:1, :], brow[:])
    nc.gpsimd.partition_broadcast(bias_bc[:], bias_bc[:1, :])

    for og in range(2, O_TILES):
        nc.gpsimd.dma_start(wb[ts(og, 512), :], w[ts(og, 512), :])
    for sg in range(2, SG):
        nc.gpsimd.dma_start(xb[ts(sg, 512), :], x[ts(sg, 512), :])

    # filtT chunks: ft[k][og][c, o'] = filtT[k*128+c, og*512+o']
    ft = [[ftp.tile([P, 512], MAIN_DT, tag=f"ft{k}_{og}", name=f"ft{k}_{og}")
           for og in range(O_TILES)] for k in range(NBLK)]

    with (
        tc.tile_pool(name="nper", bufs=1) as nper,     # per-set persistents
        tc.tile_pool(name="nx", bufs=1) as nxp,        # per-set iterates
        tc.tile_pool(name="nu", bufs=3) as nup,        # U temp
        tc.tile_pool(name="misc", bufs=1) as misc,
        tc.tile_pool(name="wtsp", bufs=2) as wtsp,     # W^T og ring
        tc.tile_pool(name="xt01", bufs=1) as xt01p,    # x^T sg0-2 resident
        tc.tile_pool(name="xtp", bufs=2) as xtp,       # x^T sg3-7 ring
        tc.tile_pool(name="obp", bufs=3) as obp,       # out staging
        tc.tile_pool(name="ps_g", bufs=4, space="PSUM") as ps_g,
        tc.tile_pool(name="ps_out", bufs=4, space="PSUM") as ps_out,
    ):
        # ---- W^T / x^T via DMA-xbar transposed loads ----
        wts = {}

        def emit_wT(og):
            wts[og] = wtsp.tile([P, NBLK, 512], MAIN_DT, tag="wts",
                                name=f"wts{og}")
            nc.sync.dma_start(wts[og][:], wb[ts(og, 512), :], transpose=True)

        xts = {}

        def emit_xt(sg):
            pool, tag = (xt01p, f"xt{sg}") if sg < 3 else (xtp, "xt")
            t = pool.tile([P, NBLK, 512], MAIN_DT, tag=tag, name=f"xt{sg}")
            nc.sync.dma_start(t[:], xb[ts(sg, 512), :], transpose=True)
            xts[sg] = t

        # ------- rotations + S prep (rmt freed right after) -------
        s_s = []
        with tc.tile_pool(name="rmt", bufs=1) as rmtp:
            rmt = rmtp.tile([P, NROT, P], F32)
            nc.sync.dma_start(rmt[:], rmt_d)
            emit_wT(0)
            emit_xt(0)
            emit_wT(1)
            emit_xt(1)
            for s in range(NSETS):
                tpg = ps_g.tile([P, SETB, P], F32, tag="g")
                for j in range(SETB):
                    nc.tensor.transpose(tpg[:, j, :], rmt[:, s * SETB + j, :],
                                        ident32[:])
                sset = nper.tile([P, SETB, P], F32, tag=f"s{s}", name=f"s{s}")
                for j in range(SETB):
                    sub(sset[:, j, :], rmt[:, s * SETB + j, :], tpg[:, j, :])
                smul(sset[:], sset[:], 0.5)              # S
                s_s.append(sset)

        # ---------- Newton-Cayley: 6 interleaved sets of 4 blocks ----------
        p32_s, p16_s, x_s = [], [], []
        for s in range(NSETS):
            g = ps_g.tile([P, SETB, P], F32, tag="g")
            for j in range(SETB):                        # S^T @ S = -S^2
                nc.tensor.matmul(g[:, j, :], lhsT=s_s[s][:, j, :],
                                 rhs=s_s[s][:, j, :], start=True, stop=True)
            p32s = nper.tile([P, SETB, P], F32, tag=f"p32{s}", name=f"p32{s}")
            add(p32s[:], bc(eI2), g[:])                  # P = (1+e)^2 I - S^2
            p16s = nper.tile([P, SETB, P], F16, tag=f"p16{s}", name=f"p16{s}")
            scp(p16s[:], p32s[:])
            xset = nxp.tile([P, SETB, P], F16, tag=f"x{s}", name=f"x{s}_init")
            smul(xset[:], p32s[:], float(X0_B))          # X0 = aI + bP
            add(xset[:], xset[:], bc(aI0))
            # fold B^T = eI12 + (2+e)S - P into the S tile now
            nc.vector.tensor_scalar(s_s[s][:], s_s[s][:], float(2.0 + EPS),
                                    None, mybir.AluOpType.mult)
            add(s_s[s][:], s_s[s][:], bc(eI12))
            sub(s_s[s][:], s_s[s][:], p32s[:])
            p32_s.append(p32s)
            p16_s.append(p16s)
            x_s.append(xset)

        for i in range(NEWTON_F16):
            do_sym = i in SYM_ITERS
            for s in range(NSETS):
                # one PSUM tile per (iter, set): g1, then g2 overwrites it
                # (the DVE read of g1 gates the g2 matmuls anyway).
                g = ps_g.tile([P, SETB, P], F32, tag="g")
                for j in range(SETB):
                    nc.tensor.matmul(g[:, j, :], lhsT=p16_s[s][:, j, :],
                                     rhs=x_s[s][:, j, :], start=True,
                                     stop=True)
                u = nup.tile([P, SETB, P], F16, tag="u")
                sub(u[:], bc(twoI), g[:])                # U = 2I - P X (DVE)
                for j in range(SETB):                    # X' = X U
                    nc.tensor.matmul(g[:, j, :], lhsT=x_s[s][:, j, :],
                                     rhs=u[:, j, :], start=True, stop=True)
                xset = nxp.tile([P, SETB, P], F16, tag=f"x{s}",
                                name=f"x{s}_{i}")
                if not do_sym:
                    if s == 0:
                        cp(xset[:], g[:])                # DVE
                    else:
                        scp(xset[:], g[:])               # ScalarE
                else:
                    xc = misc.tile([P, SETB, P], F32, tag="xc")
                    cp(xc[:], g[:])
                    tpg = ps_g.tile([P, SETB, P], F32, tag="g")
                    for j in range(SETB):
                        nc.tensor.transpose(tpg[:, j, :], xc[:, j, :],
                                            ident32[:])
                    add(xc[:], xc[:], tpg[:])
                    nc.scalar.activation(xset[:], xc[:],
                                         mybir.ActivationFunctionType.Copy,
                                         scale=0.5)
                x_s[s] = xset

        # fp32 polish + Q for a pair of sets (q first, then k, then v)
        q_s = [None] * NSETS

        def polish_q(pair):
            xfs = {}
            for s in pair:
                xf = nxp.tile([P, SETB, P], F32, tag=f"xf{s % 2}",
                              name=f"xf{s}")
                if s % 2 == 0:
                    cp(xf[:], x_s[s][:])
                else:
                    scp(xf[:], x_s[s][:])
                xfs[s] = xf
            for i in range(NEWTON_F32):
                for s in pair:
                    g = ps_g.tile([P, SETB, P], F32, tag="g")
                    for j in range(SETB):
                        nc.tensor.matmul(g[:, j, :], lhsT=p32_s[s][:, j, :],
                                         rhs=xfs[s][:, j, :], start=True,
                                         stop=True)
                    uf = misc.tile([P, SETB, P], F32, tag=f"uf{s % 2}")
                    sub(uf[:], bc(twoI), g[:])
                    for j in range(SETB):
                        nc.tensor.matmul(g[:, j, :], lhsT=xfs[s][:, j, :],
                                         rhs=uf[:, j, :], start=True,
                                         stop=True)
                    xf2 = nxp.tile([P, SETB, P], F32, tag=f"xf{s % 2}",
                                   name=f"xf{s}_{i}")
                    if s % 2 == 0:
                        cp(xf2[:], g[:])
                    else:
                        scp(xf2[:], g[:])
                    xfs[s] = xf2
            for s in pair:
                # Q = B @ X with B^T = eI12 + (2+e)S - P (folded into s_s)
                g = ps_g.tile([P, SETB, P], F32, tag="g")
                for j in range(SETB):
                    nc.tensor.matmul(g[:, j, :], lhsT=s_s[s][:, j, :],
                                     rhs=xfs[s][:, j, :], start=True,
                                     stop=True)
                qset = qpool.tile([P, SETB, P], MAIN_DT, tag=f"q{s}",
                                  name=f"q{s}")
                if s % 2 == 0:
                    cp(qset[:], g[:])
                else:
                    scp(qset[:], g[:])
                q_s[s] = qset

        def q_lhsT(n):
            return q_s[n // SETB][:, n % SETB, :]

        # ---- filtT chunks: ft[k][og] = Q^T W^T ----
        def emit_ft(og):
            part = og // 2             # q/k/v
            for k in range(NBLK):
                fg = ps_out.tile([P, 512], F32, tag="po", name=f"fg{og}_{k}")
                nc.tensor.matmul(fg[:], lhsT=q_lhsT(part * NBLK + k),
                                 rhs=wts[og][:, k, :], start=True, stop=True)
                if k % 2 == 0:
                    cp(ft[k][og][:], fg[:])
                else:
                    scp(ft[k][og][:], fg[:])

        # ---- main GEMM bursts ----
        def emit_gemm(mt, og_lo, n_og):
            sg, sb = mt // 4, mt % 4
            ob = obp.tile([P, 512 * n_og], F32, tag="ob",
                          name=f"ob{mt}_{og_lo}")
            for h in range(n_og):
                og = og_lo + h
                po = ps_out.tile([P, 512], F32, tag="po", name=f"po{mt}_{og}")
                for k in range(NBLK):
                    nc.tensor.matmul(po[:], lhsT=xts[sg][:, k, ts(sb, P)],
                                     rhs=ft[k][og][:],
                                     start=(k == 0), stop=(k == NBLK - 1))
                add(ob[:, ts(h, 512)], po[:], bias_bc[:, ts(og, 512)])
            nc.scalar.dma_start(
                out[ts(mt, P), ds(og_lo * 512, 512 * n_og)], ob[:])

        # ================= tail emission order (= priority) =============
        polish_q((0, 1))
        emit_ft(0)
        emit_ft(1)
        emit_xt(2)
        # phase 1: og0 then og1 over the first N1 row tiles; k/v polish and
        # og2-5 filtT builds slot into the stream between chunks.
        for mt in range(N1):
            emit_gemm(mt, 0, 1)
            if mt == 0:
                polish_q((2, 3))
            if mt == 2:
                emit_wT(2)
                emit_ft(2)
            if mt == 5:
                emit_wT(3)
                emit_ft(3)
        emit_xt(3)
        for mt in range(N1):
            emit_gemm(mt, 1, 1)
            if mt == 0:
                polish_q((4, 5))
            if mt == 2:
                emit_wT(4)
                emit_ft(4)
            if mt == 5:
                emit_wT(5)
                emit_ft(5)
        emit_xt(4)
        # phase 2: remaining row tiles, all og groups
        for mt in range(N1, M_TILES):
            if mt % 4 == 0 and mt + 8 < M_TILES:
                emit_xt((mt + 8) // 4)      # prefetch sg two groups ahead
            for pair in range(3):
                emit_gemm(mt, 2 * pair, 2)
        # phase 3: og2-5 for the first N1 row tiles (x^T still resident)
        for mt in range(N1):
            emit_gemm(mt, 2, 2)
            emit_gemm(mt, 4, 2)


def build():
    if "nc" in _CACHE:
        return _CACHE["nc"]
    import contextlib

    nc = bacc.Bacc("TRN2", target_bir_lowering=False, debug=False)
    with tile.TileContext(nc) as tc:
        with contextlib.ExitStack() as ctx:
            build_body(ctx, tc)
    nc.compile()
    _CACHE["nc"] = nc
    return nc


_CACHE = {}


def make_in_maps(attn_weight, bias, x, q_R, k_R, v_R):
    rmat = np.concatenate([q_R, k_R, v_R], axis=0).astype(np.float32)
    rmt = np.ascontiguousarray(rmat.transpose(1, 0, 2))  # [P, NROT, P]
    w = np.ascontiguousarray(attn_weight, dtype=np.float32)
    b = np.ascontiguousarray(bias, dtype=np.float32)
    return [
        {"x": np.ascontiguousarray(x[c], dtype=np.float32),
         "w": w, "bias": b, "rmt": rmt}
        for c in range(N_CORES)
    ]


def kernel(attn_weight, bias, x, q_R, k_R, v_R, **run_kwargs):
    nc = build()
    in_maps = make_in_maps(attn_weight, bias, x, q_R, k_R, v_R)
    res = run_bass_kernel_spmd(nc, in_maps, core_ids=list(range(N_CORES)),
                               **run_kwargs)
    out = np.stack([res.results[c]["out"] for c in range(N_CORES)], axis=0)
    _CACHE["last_results"] = res
    return out


# revision 35
# speedup vs baseline: 1.0908x; 1.0496x over previous
"""Trainium2 Bass kernel for EnhancedOFTQKVLayer.

Computes out[b,s,o] = x[b,s,:] @ filt[o,:]^T + bias[o] where
filt = [Wq @ BD(cayley(q_R)); Wk @ BD(cayley(k_R)); Wv @ BD(cayley(v_R))]
(BD = block-diagonal, cayley(A) = (I-S) inv(I+S+eps I), S = 0.5(A-A^T)).

Distribution: data-parallel - batch b (8 rows) sharded one per NeuronCore;
attn_weight / bias / rotation matrices replicated.

Per-core schedule:
  1. ALL matmul-operand transposes are done by DMA, not compute engines:
     x and W are cast fp32->bf16 straight in DRAM by SWDGE cast-DMAs
     (gpsimd queue, ordered by deadline) and pulled into SBUF
     pre-transposed by 1 MB DMA-xbar loads.  Zero PE/DVE/ScalarE cost.
  2. Cayley via SPD Newton-Schulz on P = (1+eps)^2 I - S^2 (iterates are
     polynomials in S^2, hence symmetric -> lhsT=operand works without
     transposes).  X0 = aI + bP with (a, b) the true minimax-residual
     linear init on the measured spectrum [1, 254] - one fewer fp16
     iteration than the generic init for the same residual.  5 fp16
     iterations (symmetrize on 2 and 4) + 2 fp32 polish iterations; the
     polish runs pairwise q->k->v so the q-projection finishes first and
     the polish of k/v overlaps the phase-1 GEMM.  rmat is pre-permuted
     on the host so one contiguous DMA loads all rotation blocks.
  3. Main matmul in bf16 (fp32 PSUM), three phases: (1) og0 then og1 over
     the first 12 row tiles, interleaved with the k/v polish and the
     og2-5 filtT builds, (2) row-tile-outer over the remaining 20 tiles x
     all 6 og groups (x^T streams through a 2-slot ring), (3) og2-5 for
     the first 12 row tiles (pure GEMM tail, x^T sg0-2 still resident).
     Fused bias add on DVE, 512 KB output DMAs on the ACT queue.
"""

import numpy as np

import concourse.bass as bass
import concourse.mybir as mybir
import concourse.tile as tile
from concourse import bacc
from concourse.bass import ds, ts
from concourse.masks import make_identity
from concourse.bass_utils import run_bass_kernel_spmd

F32 = mybir.dt.float32
F16 = mybir.dt.float16
BF16 = mybir.dt.bfloat16

MAIN_DT = BF16           # dtype of the big matmul inputs (x, filtT)

HIDDEN = 1024
OUT_DIM = 3 * HIDDEN
SEQ = 4096
P = 128
NBLK = 8                 # 128-blocks per hidden
NROT = 24                # 3 * NBLK rotation blocks
EPS = 1e-6
N_CORES = 8

NSETS = 6                # Newton processes blocks in sets of 4
SETB = 4

# Newton-Schulz schedule (validated offline against the jax reference:
# max block rel-err 5.0e-3 vs 8.8e-3 for the older 7+1 schedule).
NEWTON_F16 = 5
NEWTON_F32 = 2
SYM_ITERS = {4}          # symmetrize on these fp16 iterations
X0_A = 3.0874517e-02     # X0 = aI + bP (minimax residual on [1, 254])
X0_B = -1.2101700e-04

M_TILES = SEQ // P       # 32 row tiles of 128
SG = SEQ // 512          # 8 row groups of 512 (4 row tiles each)
O_TILES = OUT_DIM // 512  # 6
N1 = 12                  # phase-1 row tiles (og0/og1 early wave)


def build_body(ctx, tc):
    nc = tc.nc

    x = nc.dram_tensor("x", [SEQ, HIDDEN], F32, kind="ExternalInput").ap()
    w = nc.dram_tensor("w", [OUT_DIM, HIDDEN], F32, kind="ExternalInput").ap()
    bias = nc.dram_tensor("bias", [OUT_DIM], F32, kind="ExternalInput").ap()
    # host-side pre-permuted rotations: rmt[p, n, c] = rmat[n, p, c]
    rmt_d = nc.dram_tensor("rmt", [P, NROT, P], F32, kind="ExternalInput").ap()
    out = nc.dram_tensor("out", [SEQ, OUT_DIM], F32, kind="ExternalOutput").ap()

    sub = nc.vector.tensor_sub
    add = nc.vector.tensor_add
    smul = nc.vector.tensor_scalar_mul
    cp = nc.vector.tensor_copy
    scp = nc.scalar.copy

    def bc(t):  # broadcast a [P, P] constant over a set's middle dim
        return t[:].unsqueeze(1).to_broadcast([P, SETB, P])

    # ---- persistent pools ----
    const = ctx.enter_context(tc.tile_pool(name="const", bufs=1))
    ftp = ctx.enter_context(tc.tile_pool(name="ftp", bufs=1))
    qpool = ctx.enter_context(tc.tile_pool(name="qpool", bufs=1))
    dram = ctx.enter_context(tc.tile_pool(name="dram", bufs=1, space="DRAM"))

    ident32 = const.tile([P, P], F32)
    make_identity(nc, ident32)
    eI2 = const.tile([P, P], F32)       # (1+eps)^2 I
    smul(eI2[:], ident32[:], float((1.0 + EPS) ** 2))
    eI12 = const.tile([P, P], F32)      # ((1+eps) + (1+eps)^2) I
    smul(eI12[:], ident32[:], float((1.0 + EPS) + (1.0 + EPS) ** 2))
    twoI = const.tile([P, P], F32)      # 2 I
    smul(twoI[:], ident32[:], 2.0)
    aI0 = const.tile([P, P], F16)       # X0_A * I  (Newton init)
    smul(aI0[:], ident32[:], float(X0_A))

    # bf16 DRAM scratch (filled by SWDGE cast-DMAs, deadline order)
    xb = dram.tile([SEQ, HIDDEN], MAIN_DT)
    wb = dram.tile([OUT_DIM, HIDDEN], MAIN_DT)

    nc.gpsimd.dma_start(wb[ts(0, 512), :], w[ts(0, 512), :])
    nc.gpsimd.dma_start(xb[ts(0, 512), :], x[ts(0, 512), :])
    nc.gpsimd.dma_start(wb[ts(1, 512), :], w[ts(1, 512), :])
    nc.gpsimd.dma_start(xb[ts(1, 512), :], x[ts(1, 512), :])

    bias_bc = const.tile([P, OUT_DIM], MAIN_DT)
    with tc.tile_pool(name="biasld", bufs=1) as bl:
        brow = bl.tile([1, OUT_DIM], F32)
        nc.sync.dma_start(brow[:], bias.unsqueeze(0))
        cp(bias_bc[:1, :], brow[:])
    nc.gpsimd.partition_broadcast(bias_bc[:], bias_bc[:1, :])

    for og in range(2, O_TILES):
        nc.gpsimd.dma_start(wb[ts(og, 512), :], w[ts(og, 512), :])
    for sg in range(2, SG):
        nc.gpsimd.dma_start(xb[ts(sg, 512), :], x[ts(sg, 512), :])

    # filtT chunks: ft[k][og][c, o'] = filtT[k*128+c, og*512+o']
    ft = [[ftp.tile([P, 512], MAIN_DT, tag=f"ft{k}_{og}", name=f"ft{k}_{og}")
           for og in range(O_TILES)] for k in range(NBLK)]

    with (
        tc.tile_pool(name="nper", bufs=1) as nper,     # per-set persistents
        tc.tile_pool(name="nx", bufs=1) as nxp,        # per-set iterates
        tc.tile_pool(name="nu", bufs=3) as nup,        # U temp
        tc.tile_pool(name="misc", bufs=1) as misc,
        tc.tile_pool(name="wtsp", bufs=2) as wtsp,     # W^T og ring
        tc.tile_pool(name="xt01", bufs=1) as xt01p,    # x^T sg0-2 resident
        tc.tile_pool(name="xtp", bufs=2) as xtp,       # x^T sg3-7 ring
        tc.tile_pool(name="obp", bufs=3) as obp,       # out staging
        tc.tile_pool(name="ps_g", bufs=5, space="PSUM") as ps_g,
        tc.tile_pool(name="ps_out", bufs=3, space="PSUM") as ps_out,
    ):
        # ---- W^T / x^T via DMA-xbar transposed loads ----
        wts = {}

        def emit_wT(og):
            wts[og] = wtsp.tile([P, NBLK, 512], MAIN_DT, tag="wts",
                                name=f"wts{og}")
            nc.sync.dma_start(wts[og][:], wb[ts(og, 512), :], transpose=True)

        xts = {}

        def emit_xt(sg):
            pool, tag = (xt01p, f"xt{sg}") if sg < 3 else (xtp, "xt")
            t = pool.tile([P, NBLK, 512], MAIN_DT, tag=tag, name=f"xt{sg}")
            nc.sync.dma_start(t[:], xb[ts(sg, 512), :], transpose=True)
            xts[sg] = t

        # ------- rotations + S prep (rmt freed right after) -------
        s_s = []
        with tc.tile_pool(name="rmt", bufs=1) as rmtp:
            rmt = rmtp.tile([P, NROT, P], F32)
            nc.sync.dma_start(rmt[:], rmt_d)
            emit_wT(0)
            emit_xt(0)
            emit_wT(1)
            emit_xt(1)
            for s in range(NSETS):
                tpg = ps_g.tile([P, SETB, P], F32, tag="g")
                for j in range(SETB):
                    nc.tensor.transpose(tpg[:, j, :], rmt[:, s * SETB + j, :],
                                        ident32[:])
                sset = nper.tile([P, SETB, P], F32, tag=f"s{s}", name=f"s{s}")
                for j in range(SETB):
                    # Su = A - A^T (unscaled; the 1/2 folds into S^2, B^T)
                    sub(sset[:, j, :], rmt[:, s * SETB + j, :], tpg[:, j, :])
                s_s.append(sset)

        # ---------- Newton-Cayley: 6 interleaved sets of 4 blocks ----------
        stt = nc.vector.scalar_tensor_tensor
        p32_s, p16_s, x_s = [], [], []
        for s in range(NSETS):
            g = ps_g.tile([P, SETB, P], F32, tag="g")
            for j in range(SETB):                        # S^T @ S = -S^2
                nc.tensor.matmul(g[:, j, :], lhsT=s_s[s][:, j, :],
                                 rhs=s_s[s][:, j, :], start=True, stop=True)
            p32s = nper.tile([P, SETB, P], F32, tag=f"p32{s}", name=f"p32{s}")
            # P = (1+e)^2 I - S^2 = 0.25 (Su^T Su) + (1+e)^2 I
            stt(p32s[:], g[:], 0.25, bc(eI2),
                mybir.AluOpType.mult, mybir.AluOpType.add)
            p16s = nper.tile([P, SETB, P], F16, tag=f"p16{s}", name=f"p16{s}")
            scp(p16s[:], p32s[:])
            xset = nxp.tile([P, SETB, P], F16, tag=f"x{s}", name=f"x{s}_init")
            # X0 = aI + bP
            stt(xset[:], p32s[:], float(X0_B), bc(aI0),
                mybir.AluOpType.mult, mybir.AluOpType.add)
            # fold B^T = eI12 + ((2+e)/2) Su - P into the S tile now
            stt(s_s[s][:], s_s[s][:], float((2.0 + EPS) / 2.0), p32s[:],
                mybir.AluOpType.mult, mybir.AluOpType.subtract)
            add(s_s[s][:], s_s[s][:], bc(eI12))
            p32_s.append(p32s)
            p16_s.append(p16s)
            x_s.append(xset)

        for i in range(NEWTON_F16):
            do_sym = i in SYM_ITERS
            for s in range(NSETS):
                # one PSUM tile per (iter, set): g1, then g2 overwrites it
                # (the DVE read of g1 gates the g2 matmuls anyway).
                g = ps_g.tile([P, SETB, P], F32, tag="g")
                for j in range(SETB):
                    nc.tensor.matmul(g[:, j, :], lhsT=p16_s[s][:, j, :],
                                     rhs=x_s[s][:, j, :], start=True,
                                     stop=True)
                u = nup.tile([P, SETB, P], F16, tag="u")
                sub(u[:], bc(twoI), g[:])                # U = 2I - P X (DVE)
                for j in range(SETB):                    # X' = X U
                    nc.tensor.matmul(g[:, j, :], lhsT=x_s[s][:, j, :],
                                     rhs=u[:, j, :], start=True, stop=True)
                xset = nxp.tile([P, SETB, P], F16, tag=f"x{s}",
                                name=f"x{s}_{i}")
                if not do_sym:
                    scp(xset[:], g[:])                   # ScalarE
                else:
                    xc = misc.tile([P, SETB, P], F32, tag="xc")
                    scp(xc[:], g[:])
                    tpg = ps_g.tile([P, SETB, P], F32, tag="g")
                    for j in range(SETB):
                        nc.tensor.transpose(tpg[:, j, :], xc[:, j, :],
                                            ident32[:])
                    add(xc[:], xc[:], tpg[:])
                    nc.scalar.activation(xset[:], xc[:],
                                         mybir.ActivationFunctionType.Copy,
                                         scale=0.5)
                x_s[s] = xset

        # fp32 polish + Q for a pair of sets (q first, then k, then v)
        q_s = [None] * NSETS

        def polish_q(pair):
            xfs = {}
            for s in pair:
                xf = nxp.tile([P, SETB, P], F32, tag=f"xf{s % 2}",
                              name=f"xf{s}")
                if s % 2 == 0:
                    cp(xf[:], x_s[s][:])
                else:
                    scp(xf[:], x_s[s][:])
                xfs[s] = xf
            for i in range(NEWTON_F32):
                for s in pair:
                    g = ps_g.tile([P, SETB, P], F32, tag="g")
                    for j in range(SETB):
                        nc.tensor.matmul(g[:, j, :], lhsT=p32_s[s][:, j, :],
                                         rhs=xfs[s][:, j, :], start=True,
                                         stop=True)
                    uf = misc.tile([P, SETB, P], F32, tag=f"uf{s % 2}")
                    sub(uf[:], bc(twoI), g[:])
                    for j in range(SETB):
                        nc.tensor.matmul(g[:, j, :], lhsT=xfs[s][:, j, :],
                                         rhs=uf[:, j, :], start=True,
                                         stop=True)
                    xf2 = nxp.tile([P, SETB, P], F32, tag=f"xf{s % 2}",
                                   name=f"xf{s}_{i}")
                    if s % 2 == 0:
                        cp(xf2[:], g[:])
                    else:
                        scp(xf2[:], g[:])
                    xfs[s] = xf2
            for s in pair:
                # Q = B @ X with B^T = eI12 + (2+e)S - P (folded into s_s)
                g = ps_g.tile([P, SETB, P], F32, tag="g")
                for j in range(SETB):
                    nc.tensor.matmul(g[:, j, :], lhsT=s_s[s][:, j, :],
                                     rhs=xfs[s][:, j, :], start=True,
                                     stop=True)
                qset = qpool.tile([P, SETB, P], MAIN_DT, tag=f"q{s}",
                                  name=f"q{s}")
                if s % 2 == 0:
                    cp(qset[:], g[:])
                else:
                    scp(qset[:], g[:])
                q_s[s] = qset

        def q_lhsT(n):
            return q_s[n // SETB][:, n % SETB, :]

        # ---- filtT chunks: ft[k][og] = Q^T W^T ----
        def emit_ft(og):
            part = og // 2             # q/k/v
            for k in range(NBLK):
                fg = ps_out.tile([P, 512], F32, tag="po", name=f"fg{og}_{k}")
                nc.tensor.matmul(fg[:], lhsT=q_lhsT(part * NBLK + k),
                                 rhs=wts[og][:, k, :], start=True, stop=True)
                if k % 2 == 0:
                    cp(ft[k][og][:], fg[:])
                else:
                    scp(ft[k][og][:], fg[:])

        # ---- main GEMM bursts ----
        def emit_gemm(mt, og_lo, n_og):
            sg, sb = mt // 4, mt % 4
            ob = obp.tile([P, 512 * n_og], F32, tag="ob",
                          name=f"ob{mt}_{og_lo}")
            for h in range(n_og):
                og = og_lo + h
                po = ps_out.tile([P, 512], F32, tag="po", name=f"po{mt}_{og}")
                for k in range(NBLK):
                    nc.tensor.matmul(po[:], lhsT=xts[sg][:, k, ts(sb, P)],
                                     rhs=ft[k][og][:],
                                     start=(k == 0), stop=(k == NBLK - 1))
                add(ob[:, ts(h, 512)], po[:], bias_bc[:, ts(og, 512)])
            nc.scalar.dma_start(
                out[ts(mt, P), ds(og_lo * 512, 512 * n_og)], ob[:])

        # ================= tail emission order (= priority) =============
        polish_q((0, 1))
        emit_ft(0)
        emit_ft(1)
        emit_xt(2)
        # phase 1: og0 then og1 over the first N1 row tiles; k/v polish and
        # og2-5 filtT builds slot into the stream between chunks.
        for mt in range(N1):
            emit_gemm(mt, 0, 1)
            if mt == 0:
                polish_q((2, 3))
            if mt == 2:
                emit_wT(2)
                emit_ft(2)
            if mt == 5:
                emit_wT(3)
                emit_ft(3)
        emit_xt(3)
        for mt in range(N1):
            emit_gemm(mt, 1, 1)
            if mt == 0:
                polish_q((4, 5))
            if mt == 2:
                emit_wT(4)
                emit_ft(4)
            if mt == 5:
                emit_wT(5)
                emit_ft(5)
        emit_xt(4)
        # phase 2: remaining row tiles, all og groups
        for mt in range(N1, M_TILES):
            if mt % 4 == 0 and mt + 8 < M_TILES:
                emit_xt((mt + 8) // 4)      # prefetch sg two groups ahead
            for pair in range(3):
                emit_gemm(mt, 2 * pair, 2)
        # phase 3: og2-5 for the first N1 row tiles (x^T still resident)
        for mt in range(N1):
            emit_gemm(mt, 2, 2)
            emit_gemm(mt, 4, 2)


def build():
    if "nc" in _CACHE:
        return _CACHE["nc"]
    import contextlib

    nc = bacc.Bacc("TRN2", target_bir_lowering=False, debug=False)
    with tile.TileContext(nc) as tc:
        with contextlib.ExitStack() as ctx:
            build_body(ctx, tc)
    nc.compile()
    _CACHE["nc"] = nc
    return nc


_CACHE = {}


def make_in_maps(attn_weight, bias, x, q_R, k_R, v_R):
    rmat = np.concatenate([q_R, k_R, v_R], axis=0).astype(np.float32)
    rmt = np.ascontiguousarray(rmat.transpose(1, 0, 2))  # [P, NROT, P]
    w = np.ascontiguousarray(attn_weight, dtype=np.float32)
    b = np.ascontiguousarray(bias, dtype=np.float32)
    return [
        {"x": np.ascontiguousarray(x[c], dtype=np.float32),
         "w": w, "bias": b, "rmt": rmt}
        for c in range(N_CORES)
    ]


def kernel(attn_weight, bias, x, q_R, k_R, v_R, **run_kwargs):
    nc = build()
    in_maps = make_in_maps(attn_weight, bias, x, q_R, k_R, v_R)
    res = run_bass_kernel_spmd(nc, in_maps, core_ids=list(range(N_CORES)),
                               **run_kwargs)
    out = np.stack([res.results[c]["out"] for c in range(N_CORES)], axis=0)
    _CACHE["last_results"] = res
    return out
